# revision 1
# baseline (speedup 1.0000x reference)
"""GCN 2-layer encoder on 8 TRN2 NeuronCores (Bass/Tile).

Math (PyG GCNConv, symmetric normalization, self-loops, deg from dst):
    out1 = relu(Dh @ A @ Dh @ (x @ W1) + b1),  Dh = diag(deg^-1/2)
    out  = Dh @ A @ Dh @ (relu1 @ W2) + b2

Factorization used here (per layer):
    table = Dh @ (feat @ W)          # per-node rows, built on device
    agg[d] = sum_{e: src->d} table[src]   (self loops included as edges)
    out[d] = dinv[d] * agg[d] + b

Sharding: nodes are assigned to 8 cores (balanced by in-degree). Each core
aggregates only its own dst nodes. Aggregation is a sigma-matrix (multi-hot
lane->column) matmul accumulating in PSUM: edges of each dst are packed into
one or more SBUF "lanes"; gathered message chunks [128 lanes, F] are
multiplied by a per-tile constant sigma [128 lanes, 128 cols] on the PE.

Messages are fetched with the SWDGE dma_gather instruction (int16 indices).
Since indices are int16, the node table is split in two blocks (cores 0-3 /
cores 4-7) and each (tile, block) run is a separate gather call.

Layer-1 tables are built replicated on every core; the layer-2 table is
built sharded and exchanged with one AllGather.
"""

import sys
import types

sys.path.insert(0, "/opt/trn_rl_repo")

import numpy as np

# Register the NTFF profile hook the container's antenv stub lacks, so
# BASS_TRACE=1 profiling works under axon (harmless otherwise).
if "antenv.axon_hooks" not in sys.modules:
    try:
        from trn_agent_boot.trn_boot import _ntff_profile_via_ctypes

        _hook = _ntff_profile_via_ctypes("/opt/axon/libaxon_pjrt.so")
    except Exception:
        _hook = None
    _m = types.ModuleType("antenv.axon_hooks")
    _m.get_axon_ntff_profile_hook = lambda: _hook
    sys.modules["antenv.axon_hooks"] = _m

N = 50000
E = 800000
IN_CH = 128
HID = 128
OUT_CH = 64
NCORES = 8
P = 128
CAP = 12  # max edges per lane per block-side
GSZ = 4  # tiles per gather call group
CALL_CAP = 8  # max chunks (x128 idxs) per dma_gather call; larger calls fail on HW
SWDGE_QUEUES = 1  # SWDGE queues to spread gather desc-gen over

_CACHE = {}
LAST_RESULTS = None


# ----------------------------------------------------------------------------
# Host-side planning
# ----------------------------------------------------------------------------
def _plan(edge_index):
    src = np.asarray(edge_index[0], dtype=np.int64)
    dst = np.asarray(edge_index[1], dtype=np.int64)
    loops = np.arange(N, dtype=np.int64)
    src_all = np.concatenate([src, loops])
    dst_all = np.concatenate([dst, loops])
    deg = np.bincount(dst_all, minlength=N)
    dinv = (1.0 / np.sqrt(deg.astype(np.float64))).astype(np.float32)

    # --- node -> core (snake over degree-sorted nodes: balances sum(deg)) ---
    order = np.argsort(-deg, kind="stable")
    snake = np.tile(
        np.concatenate([np.arange(NCORES), np.arange(NCORES - 1, -1, -1)]),
        N // (2 * NCORES) + 1,
    )[:N]
    core_of = np.empty(N, dtype=np.int64)
    core_of[order] = snake

    # --- per-dst A/B in-edge counts (A = src on cores 0-3) ------------------
    isA = core_of[src_all] < (NCORES // 2)
    a_cnt = np.bincount(dst_all[isA], minlength=N)
    b_cnt = np.bincount(dst_all[~isA], minlength=N)

    # --- per-core lane packing ---------------------------------------------
    n_lanes = np.maximum(
        1, np.maximum(-(-a_cnt // CAP), -(-b_cnt // CAP))
    ).astype(np.int64)

    # pack each core's nodes into tiles of <=128 lanes, heavy lanes first
    core_tiles = []  # per core: list of tiles; tile = list of node ids
    for c in range(NCORES):
        nodes = np.where(core_of == c)[0]
        la = -(-a_cnt[nodes] // n_lanes[nodes])
        lb = -(-b_cnt[nodes] // n_lanes[nodes])
        o2 = np.argsort(-(la + lb), kind="stable")
        tiles = []
        cur = []
        cur_lanes = 0
        for i in o2:
            nd = nodes[i]
            nl = n_lanes[nd]
            if cur_lanes + nl > P:
                tiles.append(cur)
                cur = []
                cur_lanes = 0
            cur.append(nd)
            cur_lanes += nl
        if cur:
            tiles.append(cur)
        core_tiles.append(tiles)

    # per-core per-tile chunk needs
    def tile_needs(tile_nodes):
        if not tile_nodes:
            return 0, 0
        nds = np.asarray(tile_nodes)
        ca = int(np.max(-(-a_cnt[nds] // n_lanes[nds])))
        cb = int(np.max(-(-b_cnt[nds] // n_lanes[nds])))
        return ca, cb

    needs = []  # [core][tile] = (ca, cb)
    for c in range(NCORES):
        ns = [tile_needs(t) for t in core_tiles[c]]
        # sort tiles by total need desc (keeps node lists aligned)
        o3 = sorted(range(len(ns)), key=lambda i: -(ns[i][0] + ns[i][1]))
        core_tiles[c] = [core_tiles[c][i] for i in o3]
        needs.append([ns[i] for i in o3])

    # global tile count: +1 guarantees an empty last tile on every core
    # (its column 127 is the guaranteed zero row used for gather padding)
    T = max(len(t) for t in core_tiles) + 1
    SLOTS = T * P
    assert (NCORES // 2) * SLOTS <= 32768, (T, SLOTS)

    CA = np.zeros(T, dtype=np.int64)
    CB = np.zeros(T, dtype=np.int64)
    for c in range(NCORES):
        for p_, (ca, cb) in enumerate(needs[c]):
            CA[p_] = max(CA[p_], ca)
            CB[p_] = max(CB[p_], cb)
    # every tile gets at least one chunk so PSUM is always initialized
    zero = (CA + CB) == 0
    CA[zero] = 1

    # --- slot assignment ----------------------------------------------------
    slot_of = np.full(N, -1, dtype=np.int64)  # slot within core
    lane0_of = np.full(N, -1, dtype=np.int64)  # first lane within tile
    col_of = np.full(N, -1, dtype=np.int64)
    tile_of = np.full(N, -1, dtype=np.int64)
    for c in range(NCORES):
        for p_, tile_nodes in enumerate(core_tiles[c]):
            lane = 0
            for col, nd in enumerate(tile_nodes):
                tile_of[nd] = p_
                col_of[nd] = col
                lane0_of[nd] = lane
                slot_of[nd] = p_ * P + col
                lane += n_lanes[nd]
            assert lane <= P
    pos_of = core_of * SLOTS + slot_of  # global table position

    # --- CSR of edges grouped by (dst, side) -------------------------------
    side = (~isA).astype(np.int64)
    eorder = np.argsort(dst_all * 2 + side, kind="stable")
    src_pos_sorted = pos_of[src_all[eorder]].astype(np.int64)
    estart = np.zeros(N + 1, dtype=np.int64)
    np.cumsum(deg, out=estart[1:])

    # --- gather index arrays + sigma ---------------------------------------
    G = -(-T // GSZ)
    groups = [list(range(g * GSZ, min((g + 1) * GSZ, T))) for g in range(G)]
    PAD = SLOTS - 1
    HALF = (NCORES // 2) * SLOTS

    tot_chunks = int(np.sum(CA) + np.sum(CB))
    idx_cores = []
    sigma_cores = []
    dinv_own_cores = []
    for c in range(NCORES):
        tiles = core_tiles[c]
        blocksA = [np.full((int(CA[p_]), P), PAD, np.int64) for p_ in range(T)]
        blocksB = [np.full((int(CB[p_]), P), PAD, np.int64) for p_ in range(T)]
        sig = np.zeros((T, P, P), dtype=np.float16)
        dvo = np.zeros((P, T), dtype=np.float32)
        for p_ in range(min(len(tiles), T)):
            for nd in tiles[p_]:
                nl = int(n_lanes[nd])
                l0 = int(lane0_of[nd])
                col = int(col_of[nd])
                sig[p_, l0 : l0 + nl, col] = 1.0
                dvo[col, p_] = dinv[nd]
                s0 = int(estart[nd])
                a = int(a_cnt[nd])
                b = int(b_cnt[nd])
                asrc = src_pos_sorted[s0 : s0 + a]
                bsrc = src_pos_sorted[s0 + a : s0 + a + b] - HALF
                for j in range(nl):
                    ach = asrc[j::nl]
                    bch = bsrc[j::nl]
                    if len(ach):
                        blocksA[p_][: len(ach), l0 + j] = ach
                    if len(bch):
                        blocksB[p_][: len(bch), l0 + j] = bch
        flat = []
        for g in groups:
            for p_ in g:
                flat.append(blocksA[p_].reshape(-1))
            for p_ in g:
                flat.append(blocksB[p_].reshape(-1))
        flat = np.concatenate(flat) if flat else np.zeros(0, np.int64)
        assert flat.size == tot_chunks * P
        assert flat.min() >= 0 and flat.max() < HALF
        wrapped = flat.astype(np.int16).reshape(-1, 16).T.copy()  # [16, n/16]
        idx_cores.append(np.tile(wrapped, (8, 1)))  # replicate to 128 parts
        sigma_cores.append(sig)
        dinv_own_cores.append(dvo)

    # dinv for the whole table (all cores' slots), [128, 8*T]
    dinv_all = np.zeros((P, NCORES * T), dtype=np.float32)
    for c in range(NCORES):
        dinv_all[:, c * T : (c + 1) * T] = dinv_own_cores[c]

    return dict(
        T=T,
        SLOTS=SLOTS,
        CA=CA,
        CB=CB,
        groups=groups,
        tot_chunks=tot_chunks,
        core_of=core_of,
        slot_of=slot_of,
        pos_of=pos_of,
        dinv=dinv,
        idx_cores=idx_cores,
        sigma_cores=sigma_cores,
        dinv_own_cores=dinv_own_cores,
        dinv_all=dinv_all,
    )


# ----------------------------------------------------------------------------
# Device kernel
# ----------------------------------------------------------------------------
def _build(T, CA, CB, groups, tot_chunks, use_gather=True, use_collective=True):
    import concourse.bass as bass
    import concourse.mybir as mybir
    import concourse.tile as tile
    from concourse import bacc

    f16 = mybir.dt.float16
    f32 = mybir.dt.float32
    i16 = mybir.dt.int16
    SLOTS = T * P
    ROWS = NCORES * SLOTS
    HALFROWS = ROWS // 2
    NT = NCORES * T
    max_ca = max(int(sum(CA[p_] for p_ in g)) for g in groups)
    max_cb = max(int(sum(CB[p_] for p_ in g)) for g in groups)

    nc = bacc.Bacc(
        "TRN2",
        target_bir_lowering=False,
        num_devices=NCORES,
        num_swdge_queues=SWDGE_QUEUES,
    )
    qn = [0]

    def _next_q():
        qn[0] = (qn[0] + 1) % SWDGE_QUEUES
        return qn[0]

    xT_in = nc.dram_tensor("xT", [NT, P, P], f16, kind="ExternalInput")
    w1_in = nc.dram_tensor("W1", [IN_CH, HID], f16, kind="ExternalInput")
    w2_in = nc.dram_tensor("W2", [HID, OUT_CH], f16, kind="ExternalInput")
    b1_in = nc.dram_tensor("b1bc", [P, HID], f32, kind="ExternalInput")
    b2_in = nc.dram_tensor("b2bc", [P, OUT_CH], f32, kind="ExternalInput")
    id_in = nc.dram_tensor("ident", [P, P], f16, kind="ExternalInput")
    sig_in = nc.dram_tensor("sigma", [T, P, P], f16, kind="ExternalInput")
    da_in = nc.dram_tensor("dinv_all", [P, NT], f32, kind="ExternalInput")
    do_in = nc.dram_tensor("dinv_own", [P, T], f32, kind="ExternalInput")
    idx_in = nc.dram_tensor("idx", [P, tot_chunks * 8], i16, kind="ExternalInput")
    out_ext = nc.dram_tensor("out", [SLOTS, OUT_CH], f32, kind="ExternalOutput")

    with tile.TileContext(nc) as tc:
        with (
            tc.tile_pool(name="const", bufs=1) as cpool,
            tc.tile_pool(name="xt", bufs=3) as xtpool,
            tc.tile_pool(name="sig", bufs=3) as sigpool,
            tc.tile_pool(name="stg", bufs=2) as stgpool,
            tc.tile_pool(name="drain", bufs=3) as dpool,
            tc.tile_pool(name="psb", bufs=2, space="PSUM") as ps_build,
            tc.tile_pool(name="psa", bufs=2, space="PSUM") as ps_agg,
            tc.tile_pool(name="pst", bufs=2, space="PSUM") as ps_tr,
            tc.tile_pool(name="psm", bufs=2, space="PSUM") as ps_mm2,
            tc.tile_pool(name="dram", bufs=1, space="DRAM") as dram,
        ):
            # ---- constants into SBUF ----
            w1_sb = cpool.tile([IN_CH, HID], f16)
            nc.sync.dma_start(out=w1_sb[:], in_=w1_in[:])
            w2_sb = cpool.tile([HID, OUT_CH], f16)
            nc.sync.dma_start(out=w2_sb[:], in_=w2_in[:])
            b1_sb = cpool.tile([P, HID], f32)
            nc.sync.dma_start(out=b1_sb[:], in_=b1_in[:])
            b2_sb = cpool.tile([P, OUT_CH], f32)
            nc.sync.dma_start(out=b2_sb[:], in_=b2_in[:])
            id_sb = cpool.tile([P, P], f16)
            nc.sync.dma_start(out=id_sb[:], in_=id_in[:])
            da_sb = cpool.tile([P, NT], f32)
            nc.sync.dma_start(out=da_sb[:], in_=da_in[:])
            do_sb = cpool.tile([P, T], f32)
            nc.sync.dma_start(out=do_sb[:], in_=do_in[:])
            idx_sb = cpool.tile([P, tot_chunks * 8], i16)
            nc.sync.dma_start(out=idx_sb[:], in_=idx_in[:])

            table1 = dram.tile([ROWS, HID], f16)
            shard2 = dram.tile([SLOTS, P], f16)
            table2 = dram.tile([ROWS, P], f16, addr_space="Shared" if use_collective else "Local")

            # ---- phase 1: table1 = dinv * (x @ W1), full, replicated ----
            for j in range(NT):
                xt_t = xtpool.tile([P, P], f16, tag="xt")
                nc.sync.dma_start(out=xt_t[:], in_=xT_in[j])
                bps = ps_build.tile([P, HID], f32, tag="build")
                nc.tensor.matmul(
                    bps[:], lhsT=xt_t[:], rhs=w1_sb[:], start=True, stop=True
                )
                h1t = xtpool.tile([P, HID], f16, tag="h1t")
                if j % 2 == 0:
                    nc.scalar.activation(
                        h1t[:],
                        bps[:],
                        mybir.ActivationFunctionType.Copy,
                        scale=da_sb[:, j : j + 1],
                    )
                else:
                    nc.vector.tensor_scalar_mul(h1t[:], bps[:], da_sb[:, j : j + 1])
                nc.sync.dma_start(out=table1[j * P : (j + 1) * P, :], in_=h1t[:])

            # ---- per-layer aggregation ----
            def aggregate(layer):
                tab = table1 if layer == 0 else table2
                nfeat = HID if layer == 0 else OUT_CH
                coff = 0
                for g in groups:
                    ca_g = int(sum(int(CA[p_]) for p_ in g))
                    cb_g = int(sum(int(CB[p_]) for p_ in g))
                    stA = stB = None
                    if ca_g:
                        stA = stgpool.tile([P, max_ca, P], f16, tag="stgA")
                        if use_gather:
                            for s_ in range(0, ca_g, CALL_CAP):
                                n_ = min(CALL_CAP, ca_g - s_)
                                nc.gpsimd.dma_gather(
                                    stA[:, s_ : s_ + n_, :],
                                    tab[0:HALFROWS, :],
                                    idx_sb[:, (coff + s_) * 8 : (coff + s_ + n_) * 8],
                                    n_ * P,
                                    n_ * P,
                                    P,
                                    queue_num=_next_q(),
                                )
                        else:
                            nc.sync.dma_start(
                                out=stA[:, 0:ca_g, :],
                                in_=tab[0 : ca_g * P, :].rearrange(
                                    "(c p) f -> p c f", p=P
                                ),
                            )
                    if cb_g:
                        stB = stgpool.tile([P, max_cb, P], f16, tag="stgB")
                        if use_gather:
                            for s_ in range(0, cb_g, CALL_CAP):
                                n_ = min(CALL_CAP, cb_g - s_)
                                nc.gpsimd.dma_gather(
                                    stB[:, s_ : s_ + n_, :],
                                    tab[HALFROWS:ROWS, :],
                                    idx_sb[
                                        :,
                                        (coff + ca_g + s_) * 8 : (coff + ca_g + s_ + n_) * 8,
                                    ],
                                    n_ * P,
                                    n_ * P,
                                    P,
                                    queue_num=_next_q(),
                                )
                        else:
                            nc.sync.dma_start(
                                out=stB[:, 0:cb_g, :],
                                in_=tab[0 : cb_g * P, :].rearrange(
                                    "(c p) f -> p c f", p=P
                                ),
                            )
                    a_off = 0
                    b_off = 0
                    for p_ in g:
                        sg = sigpool.tile([P, P], f16, tag="sig")
                        nc.sync.dma_start(out=sg[:], in_=sig_in[p_])
                        aps = ps_agg.tile([P, nfeat], f32, tag="agg")
                        ntot = int(CA[p_]) + int(CB[p_])
                        k = 0
                        for ci in range(int(CA[p_])):
                            nc.tensor.matmul(
                                aps[:],
                                lhsT=sg[:],
                                rhs=stA[:, a_off + ci, 0:nfeat],
                                start=(k == 0),
                                stop=(k == ntot - 1),
                            )
                            k += 1
                        for ci in range(int(CB[p_])):
                            nc.tensor.matmul(
                                aps[:],
                                lhsT=sg[:],
                                rhs=stB[:, b_off + ci, 0:nfeat],
                                start=(k == 0),
                                stop=(k == ntot - 1),
                            )
                            k += 1
                        a_off += int(CA[p_])
                        b_off += int(CB[p_])
                        drain(layer, p_, aps)
                    coff += ca_g + cb_g

            def drain(layer, p_, aps):
                dv = do_sb[:, p_ : p_ + 1]
                if layer == 0:
                    # r1 = dinv*agg + b1 ; r3 = relu(r1)*dinv (fp16)
                    r1 = dpool.tile([P, HID], f32, tag="r1")
                    nc.scalar.activation(
                        r1[:], aps[:], mybir.ActivationFunctionType.Copy, scale=dv
                    )
                    nc.vector.tensor_add(r1[:], r1[:], b1_sb[:])
                    r3 = dpool.tile([P, HID], f16, tag="r3")
                    nc.vector.tensor_scalar(
                        r3[:], r1[:], 0.0, dv, mybir.AluOpType.max, mybir.AluOpType.mult
                    )
                    psT = ps_tr.tile([P, P], f16, tag="tr")
                    nc.tensor.transpose(psT[:], r3[:], id_sb[:])
                    rT = dpool.tile([P, P], f16, tag="rT")
                    nc.vector.tensor_copy(rT[:], psT[:])
                    ps2 = ps_mm2.tile([P, OUT_CH], f32, tag="mm2")
                    nc.tensor.matmul(
                        ps2[:], lhsT=rT[:], rhs=w2_sb[:], start=True, stop=True
                    )
                    t2 = dpool.tile([P, P], f16, tag="t2")
                    nc.scalar.activation(
                        t2[:, 0:OUT_CH], ps2[:], mybir.ActivationFunctionType.Copy
                    )
                    nc.vector.memset(t2[:, OUT_CH:P], 0.0)
                    nc.sync.dma_start(
                        out=shard2[p_ * P : (p_ + 1) * P, :], in_=t2[:]
                    )
                else:
                    o1 = dpool.tile([P, OUT_CH], f32, tag="o1")
                    nc.scalar.activation(
                        o1[:], aps[:], mybir.ActivationFunctionType.Copy, scale=dv
                    )
                    nc.vector.tensor_add(o1[:], o1[:], b2_sb[:])
                    nc.sync.dma_start(
                        out=out_ext[p_ * P : (p_ + 1) * P, :], in_=o1[:]
                    )

            aggregate(0)

            if use_collective:
                nc.gpsimd.collective_compute(
                    "AllGather",
                    mybir.AluOpType.bypass,
                    replica_groups=[list(range(NCORES))],
                    ins=[shard2.opt()],
                    outs=[table2.opt()],
                )
            else:
                for c_ in range(NCORES):
                    nc.sync.dma_start(
                        out=table2[c_ * SLOTS : (c_ + 1) * SLOTS, :], in_=shard2[:]
                    )

            aggregate(1)

    nc.compile()  # bacc passes: library loads, register allocation, DCE
    _split_sync_waits(nc, mybir, max_waits=1)
    return nc


def _split_sync_waits(nc, mybir, max_waits=1):
    """This walrus build rejects instructions with more than `max_waits` sync
    waits; hoist excess waits onto injected same-engine InstNoOps."""
    n_split = 0
    for fn in nc.m.functions:
        for bb in fn.blocks:
            out = []
            changed = False
            for ins in bb.instructions:
                si = ins.sync_info
                if si is not None and si.on_wait and len(si.on_wait) > max_waits:
                    waits = list(si.on_wait)
                    excess = waits[:-max_waits]
                    for i in range(0, len(excess), max_waits):
                        nop = mybir.InstNoOp(
                            name=nc.get_next_instruction_name(),
                            sync_info=mybir.SyncInfo(
                                on_wait=excess[i : i + max_waits], on_update=[]
                            ),
                            bass_nofuse=True,
                            engine=ins.engine,
                        )
                        out.append(nop)
                        n_split += 1
                    si.on_wait = waits[-max_waits:]
                    ins.sync_info = si
                    changed = True
                out.append(ins)
            if changed:
                bb.instructions = out
    return n_split


# ----------------------------------------------------------------------------
# Entry point
# ----------------------------------------------------------------------------
def kernel(x, edge_index, W1, b1, W2, b2):
    global LAST_RESULTS
    from concourse.bass_utils import run_bass_kernel_spmd

    x = np.asarray(x)
    W1a = np.asarray(W1)
    b1a = np.asarray(b1)
    W2a = np.asarray(W2)
    b2a = np.asarray(b2)

    key = hash(np.asarray(edge_index)[:, :: E // 997].tobytes())
    if key not in _CACHE:
        plan = _plan(edge_index)
        nc = _build(
            plan["T"], plan["CA"], plan["CB"], plan["groups"], plan["tot_chunks"]
        )
        _CACHE[key] = (plan, nc)
    plan, nc = _CACHE[key]

    T = plan["T"]
    SLOTS = plan["SLOTS"]
    NT = NCORES * T

    # xT in table order, tile-major: [NT, 128 infeat, 128 nodes]
    xT = np.zeros((NT, P, P), dtype=np.float16)
    nodes = np.arange(N)
    gpos = plan["pos_of"]  # global table position per node
    xTflat = np.zeros((P, NCORES * SLOTS), dtype=np.float16)
    xTflat[:, gpos] = x.astype(np.float16).T
    xT[:] = xTflat.reshape(P, NT, P).transpose(1, 0, 2)

    in_common = {
        "xT": xT,
        "W1": W1a.astype(np.float16),
        "W2": W2a.astype(np.float16),
        "b1bc": np.broadcast_to(b1a.astype(np.float32), (P, HID)).copy(),
        "b2bc": np.broadcast_to(b2a.astype(np.float32), (P, OUT_CH)).copy(),
        "ident": np.eye(P, dtype=np.float16),
        "dinv_all": plan["dinv_all"],
    }
    in_maps = []
    for c in range(NCORES):
        m = dict(in_common)
        m["sigma"] = plan["sigma_cores"][c]
        m["dinv_own"] = plan["dinv_own_cores"][c]
        m["idx"] = plan["idx_cores"][c]
        in_maps.append(m)

    res = run_bass_kernel_spmd(nc, in_maps, core_ids=list(range(NCORES)))
    LAST_RESULTS = res

    out = np.empty((N, OUT_CH), dtype=np.float32)
    core_of = plan["core_of"]
    slot_of = plan["slot_of"]
    for c in range(NCORES):
        sel = core_of == c
        out[sel] = res.results[c]["out"][slot_of[sel]]
    return out



# revision 7
# speedup vs baseline: 3.1441x; 3.1441x over previous
"""GCN 2-layer encoder on 8 TRN2 NeuronCores (Bass/Tile).

Math (PyG GCNConv, symmetric normalization, self-loops, deg from dst):
    out1 = relu(Dh @ A @ Dh @ (x @ W1) + b1),  Dh = diag(deg^-1/2)
    out  = Dh @ A @ Dh @ (relu1 @ W2) + b2

Factorization used here (per layer):
    table = Dh @ (feat @ W)          # per-node rows, built on device
    agg[d] = sum_{e: src->d} table[src]   (self loops included as edges)
    out[d] = dinv[d] * agg[d] + b

Sharding: nodes are assigned to 8 cores (balanced by in-degree). Each core
builds only its own table shard; shards are exchanged with one AllGather per
layer. Each core aggregates only its own dst nodes: edges of each dst are
packed into SBUF "lanes"; message chunks [128 lanes, F] are fetched with
SWDGE dma_gather (int16 indices, 4 queues round-robin — a single queue
backpressures descriptor generation ~3x) and multiplied by a per-tile
constant sigma [128 lanes, 128 cols] on the PE, accumulating in PSUM.

Since indices are int16, the node table is split in two halves (src cores
0-3 / 4-7) and each (tile, half) run is a separate gather call.
"""

import sys
import types

sys.path.insert(0, "/opt/trn_rl_repo")

import numpy as np

# Register the NTFF profile hook the container's antenv stub lacks, so
# BASS_TRACE=1 profiling works under axon (harmless otherwise).
if "antenv.axon_hooks" not in sys.modules:
    try:
        from trn_agent_boot.trn_boot import _ntff_profile_via_ctypes

        _hook = _ntff_profile_via_ctypes("/opt/axon/libaxon_pjrt.so")
    except Exception:
        _hook = None
    _m = types.ModuleType("antenv.axon_hooks")
    _m.get_axon_ntff_profile_hook = lambda: _hook
    sys.modules["antenv.axon_hooks"] = _m

N = 50000
E = 800000
IN_CH = 128
HID = 128
OUT_CH = 64
NCORES = 8
P = 128
CAP = 15  # max edges per lane per half
CALL_CAP = 8  # max chunks (x128 idxs) per dma_gather call; larger calls fail on HW
SWDGE_QUEUES = 4  # ucode MAX_SWDGE_QUEUES; one queue backpressures desc-gen

_CACHE = {}
LAST_RESULTS = None


# ----------------------------------------------------------------------------
# Host-side planning
# ----------------------------------------------------------------------------
def _plan(edge_index):
    src = np.asarray(edge_index[0], dtype=np.int64)
    dst = np.asarray(edge_index[1], dtype=np.int64)
    loops = np.arange(N, dtype=np.int64)
    src_all = np.concatenate([src, loops])
    dst_all = np.concatenate([dst, loops])
    deg = np.bincount(dst_all, minlength=N)
    dinv = (1.0 / np.sqrt(deg.astype(np.float64))).astype(np.float32)

    # --- node -> core (snake over degree-sorted nodes: balances sum(deg)) ---
    order = np.argsort(-deg, kind="stable")
    snake = np.tile(
        np.concatenate([np.arange(NCORES), np.arange(NCORES - 1, -1, -1)]),
        N // (2 * NCORES) + 1,
    )[:N]
    core_of = np.empty(N, dtype=np.int64)
    core_of[order] = snake

    # --- per-dst A/B in-edge counts (A = src on cores 0-3) ------------------
    isA = core_of[src_all] < (NCORES // 2)
    a_cnt = np.bincount(dst_all[isA], minlength=N)
    b_cnt = np.bincount(dst_all[~isA], minlength=N)

    # --- per-node lanes; per-half fill = chunks a node's lanes need --------
    n_lanes = np.maximum(
        1, np.maximum(-(-a_cnt // CAP), -(-b_cnt // CAP))
    ).astype(np.int64)
    af = -(-a_cnt // n_lanes)
    bf = -(-b_cnt // n_lanes)

    # pack each core's nodes into tiles of <=128 lanes, uniform fill per tile
    core_tiles = []  # per core: list of tiles; tile = list of node ids
    for c in range(NCORES):
        nodes = np.where(core_of == c)[0]
        key = np.maximum(af[nodes], bf[nodes]) * 1000 + np.minimum(
            af[nodes], bf[nodes]
        )
        o2 = np.argsort(-key, kind="stable")
        tiles = []
        cur = []
        cur_lanes = 0
        for i in o2:
            nd = nodes[i]
            nl = n_lanes[nd]
            if cur_lanes + nl > P:
                tiles.append(cur)
                cur = []
                cur_lanes = 0
            cur.append(nd)
            cur_lanes += nl
        if cur:
            tiles.append(cur)
        needs = [
            max(int(af[nd]) for nd in t) + max(int(bf[nd]) for nd in t)
            for t in tiles
        ]
        o3 = sorted(range(len(tiles)), key=lambda i: -needs[i])
        core_tiles.append([tiles[i] for i in o3])

    # global tile count: +1 guarantees an empty last tile on every core
    # (its slot 127 is the guaranteed zero row used for gather padding)
    T = max(len(t) for t in core_tiles) + 1
    SLOTS = T * P
    ROWS = NCORES * SLOTS
    HALF = (NCORES // 2) * SLOTS
    assert HALF <= 32768, (T, SLOTS)

    CA = np.zeros(T, dtype=np.int64)
    CB = np.zeros(T, dtype=np.int64)
    for c in range(NCORES):
        for p_, tile_nodes in enumerate(core_tiles[c]):
            CA[p_] = max(CA[p_], max(int(af[nd]) for nd in tile_nodes))
            CB[p_] = max(CB[p_], max(int(bf[nd]) for nd in tile_nodes))
    zero = (CA + CB) == 0
    CA[zero] = 1  # every tile gets >=1 chunk so PSUM is always initialized
    tot_chunks = int(CA.sum() + CB.sum())

    # --- slot assignment ----------------------------------------------------
    slot_of = np.full(N, -1, dtype=np.int64)
    lane0_of = np.full(N, -1, dtype=np.int64)
    col_of = np.full(N, -1, dtype=np.int64)
    for c in range(NCORES):
        for p_, tile_nodes in enumerate(core_tiles[c]):
            lane = 0
            for col, nd in enumerate(tile_nodes):
                col_of[nd] = col
                lane0_of[nd] = lane
                slot_of[nd] = p_ * P + col
                lane += n_lanes[nd]
            assert lane <= P
    pos_of = core_of * SLOTS + slot_of  # global table row per node

    # --- CSR of edges grouped by (dst, side) -------------------------------
    side = (~isA).astype(np.int64)
    eorder = np.argsort(dst_all * 2 + side, kind="stable")
    src_pos_sorted = pos_of[src_all[eorder]].astype(np.int64)
    estart = np.zeros(N + 1, dtype=np.int64)
    np.cumsum(deg, out=estart[1:])

    # --- gather index arrays + sigma ---------------------------------------
    PAD = HALF - 1  # guaranteed-zero row within each half
    idx_cores = []
    sigma_cores = []
    dinv_own_cores = []
    for c in range(NCORES):
        tiles = core_tiles[c]
        blocksA = [np.full((int(CA[p_]), P), PAD, np.int64) for p_ in range(T)]
        blocksB = [np.full((int(CB[p_]), P), PAD, np.int64) for p_ in range(T)]
        sig = np.zeros((T, P, P), dtype=np.float16)
        dvo = np.zeros((P, T), dtype=np.float32)
        for p_ in range(len(tiles)):
            for nd in tiles[p_]:
                nl = int(n_lanes[nd])
                l0 = int(lane0_of[nd])
                col = int(col_of[nd])
                sig[p_, l0 : l0 + nl, col] = 1.0
                dvo[col, p_] = dinv[nd]
                s0 = int(estart[nd])
                a = int(a_cnt[nd])
                b = int(b_cnt[nd])
                asrc = src_pos_sorted[s0 : s0 + a]
                bsrc = src_pos_sorted[s0 + a : s0 + a + b] - HALF
                for j in range(nl):
                    ach = asrc[j::nl]
                    bch = bsrc[j::nl]
                    if len(ach):
                        blocksA[p_][: len(ach), l0 + j] = ach
                    if len(bch):
                        blocksB[p_][: len(bch), l0 + j] = bch
        flat = []
        for p_ in range(T):
            flat.append(blocksA[p_].reshape(-1))
            flat.append(blocksB[p_].reshape(-1))
        flat = np.concatenate(flat)
        assert flat.size == tot_chunks * P
        assert flat.min() >= 0 and flat.max() < HALF
        wrapped = flat.astype(np.int16).reshape(-1, 16).T.copy()  # [16, n*8]
        idx_cores.append(np.tile(wrapped, (8, 1)))  # replicate to 128 parts
        sigma_cores.append(
            np.ascontiguousarray(sig.transpose(1, 0, 2).reshape(P, T * P))
        )
        dinv_own_cores.append(dvo)

    return dict(
        T=T,
        SLOTS=SLOTS,
        CA=CA,
        CB=CB,
        tot_chunks=tot_chunks,
        core_of=core_of,
        slot_of=slot_of,
        pos_of=pos_of,
        dinv=dinv,
        idx_cores=idx_cores,
        sigma_cores=sigma_cores,
        dinv_own_cores=dinv_own_cores,
    )


# ----------------------------------------------------------------------------
# Device kernel
# ----------------------------------------------------------------------------
def _build(T, CA, CB, tot_chunks):
    import concourse.bass as bass
    import concourse.mybir as mybir
    import concourse.tile as tile
    from concourse import bacc

    f16 = mybir.dt.float16
    f32 = mybir.dt.float32
    i16 = mybir.dt.int16
    SLOTS = T * P
    ROWS = NCORES * SLOTS
    HALFROWS = ROWS // 2
    CTMAX = -(-int(max(CA + CB)) // 4) * 4  # staging chunks, rounded to packs of 4

    nc = bacc.Bacc(
        "TRN2",
        target_bir_lowering=False,
        num_devices=NCORES,
        num_swdge_queues=SWDGE_QUEUES,
    )
    qn = [0]

    def _next_q():
        q = qn[0]
        qn[0] = (qn[0] + 1) % SWDGE_QUEUES
        return q

    xTs_in = nc.dram_tensor("xTs", [T, P, P], f16, kind="ExternalInput")
    w1_in = nc.dram_tensor("W1", [IN_CH, HID], f16, kind="ExternalInput")
    w2_in = nc.dram_tensor("W2", [HID, OUT_CH], f16, kind="ExternalInput")
    b1_in = nc.dram_tensor("b1bc", [P, HID], f32, kind="ExternalInput")
    b2_in = nc.dram_tensor("b2bc", [P, OUT_CH], f32, kind="ExternalInput")
    id_in = nc.dram_tensor("ident", [P, P], f16, kind="ExternalInput")
    sig_in = nc.dram_tensor("sigma", [P, T * P], f16, kind="ExternalInput")
    do_in = nc.dram_tensor("dinv_own", [P, T], f32, kind="ExternalInput")
    idx_in = nc.dram_tensor("idx", [P, tot_chunks * 8], i16, kind="ExternalInput")
    out_ext = nc.dram_tensor("out", [SLOTS, OUT_CH], f32, kind="ExternalOutput")

    with tile.TileContext(nc) as tc:
        with (
            tc.tile_pool(name="const", bufs=1) as cpool,
            tc.tile_pool(name="xt", bufs=3) as xtpool,
            tc.tile_pool(name="stg", bufs=3) as stgpool,
            tc.tile_pool(name="drain", bufs=3) as dpool,
            tc.tile_pool(name="psb", bufs=2, space="PSUM") as ps_build,
            tc.tile_pool(name="psa", bufs=2, space="PSUM") as ps_agg,
            tc.tile_pool(name="pst", bufs=2, space="PSUM") as ps_tr,
            tc.tile_pool(name="psm", bufs=2, space="PSUM") as ps_mm2,
            tc.tile_pool(name="dram", bufs=1, space="DRAM") as dram,
        ):
            # ---- constants into SBUF ----
            w1_sb = cpool.tile([IN_CH, HID], f16)
            nc.sync.dma_start(out=w1_sb[:], in_=w1_in[:])
            w2_sb = cpool.tile([HID, OUT_CH], f16)
            nc.sync.dma_start(out=w2_sb[:], in_=w2_in[:])
            b1_sb = cpool.tile([P, HID], f32)
            nc.sync.dma_start(out=b1_sb[:], in_=b1_in[:])
            b2_sb = cpool.tile([P, OUT_CH], f32)
            nc.sync.dma_start(out=b2_sb[:], in_=b2_in[:])
            id_sb = cpool.tile([P, P], f16)
            nc.sync.dma_start(out=id_sb[:], in_=id_in[:])
            sig_sb = cpool.tile([P, T * P], f16)
            nc.sync.dma_start(out=sig_sb[:], in_=sig_in[:])
            do_sb = cpool.tile([P, T], f32)
            nc.sync.dma_start(out=do_sb[:], in_=do_in[:])
            idx_sb = cpool.tile([P, tot_chunks * 8], i16)
            nc.sync.dma_start(out=idx_sb[:], in_=idx_in[:])

            shard1 = dram.tile([SLOTS, HID], f16)
            table1 = dram.tile([ROWS, HID], f16, addr_space="Shared")
            shard2 = dram.tile([SLOTS, P], f16)
            table2 = dram.tile([ROWS, P], f16, addr_space="Shared")

            # ---- phase 1: own shard of table1 = dinv * (x @ W1) ----
            for j in range(T):
                xt_t = xtpool.tile([P, P], f16, tag="xt")
                nc.sync.dma_start(out=xt_t[:], in_=xTs_in[j])
                bps = ps_build.tile([P, HID], f32, tag="build")
                nc.tensor.matmul(
                    bps[:], lhsT=xt_t[:], rhs=w1_sb[:], start=True, stop=True
                )
                h1t = xtpool.tile([P, HID], f16, tag="h1t")
                nc.scalar.activation(
                    h1t[:],
                    bps[:],
                    mybir.ActivationFunctionType.Copy,
                    scale=do_sb[:, j : j + 1],
                )
                nc.sync.dma_start(out=shard1[j * P : (j + 1) * P, :], in_=h1t[:])

            nc.gpsimd.collective_compute(
                "AllGather",
                mybir.AluOpType.bypass,
                replica_groups=[list(range(NCORES))],
                ins=[shard1.opt()],
                outs=[table1.opt()],
            )

            # ---- per-layer drain + aggregation ----
            def drain(layer, p_, aps):
                # combine the four interleaved partial sums in the wide PSUM
                dv = do_sb[:, p_ : p_ + 1]
                q = dpool.tile([P, P], f32, tag="q")
                nc.vector.tensor_copy(q[:], aps[:, 0:P])
                nc.vector.tensor_add(q[:], q[:], aps[:, P : 2 * P])
                nc.vector.tensor_add(q[:], q[:], aps[:, 2 * P : 3 * P])
                nc.vector.tensor_add(q[:], q[:], aps[:, 3 * P : 4 * P])
                if layer == 0:
                    # r1 = dinv*agg + b1 ; r3 = relu(r1)*dinv (fp16)
                    r1 = dpool.tile([P, HID], f32, tag="r1")
                    nc.scalar.activation(
                        r1[:], q[:], mybir.ActivationFunctionType.Copy, scale=dv
                    )
                    nc.vector.tensor_add(r1[:], r1[:], b1_sb[:])
                    r3 = dpool.tile([P, HID], f16, tag="r3")
                    nc.scalar.activation(
                        r3[:], r1[:], mybir.ActivationFunctionType.Relu, scale=dv
                    )
                    psT = ps_tr.tile([P, P], f16, tag="tr")
                    nc.tensor.transpose(psT[:], r3[:], id_sb[:])
                    rT = dpool.tile([P, P], f16, tag="rT")
                    nc.vector.tensor_copy(rT[:], psT[:])
                    ps2 = ps_mm2.tile([P, OUT_CH], f32, tag="mm2")
                    nc.tensor.matmul(
                        ps2[:], lhsT=rT[:], rhs=w2_sb[:], start=True, stop=True
                    )
                    t2 = dpool.tile([P, P], f16, tag="t2")
                    nc.scalar.activation(
                        t2[:, 0:OUT_CH], ps2[:], mybir.ActivationFunctionType.Copy
                    )
                    nc.vector.memset(t2[:, OUT_CH:P], 0.0)
                    nc.sync.dma_start(out=shard2[p_ * P : (p_ + 1) * P, :], in_=t2[:])
                else:
                    o1 = dpool.tile([P, OUT_CH], f32, tag="o1")
                    nc.scalar.activation(
                        o1[:], q[:, 0:OUT_CH], mybir.ActivationFunctionType.Copy, scale=dv
                    )
                    nc.vector.tensor_add(o1[:], o1[:], b2_sb[:])
                    nc.sync.dma_start(out=out_ext[p_ * P : (p_ + 1) * P, :], in_=o1[:])

            def aggregate(layer):
                tab = table1 if layer == 0 else table2
                coff = 0
                for p_ in range(T):
                    ca = int(CA[p_])
                    cb = int(CB[p_])
                    ntot = ca + cb
                    npacks = -(-ntot // 4)
                    # A chunks at [0, ca), B chunks at [ca, ca+cb)
                    stg = stgpool.tile([P, CTMAX, P], f16, tag="stg")
                    for s_ in range(0, ca, CALL_CAP):
                        n_ = min(CALL_CAP, ca - s_)
                        nc.gpsimd.dma_gather(
                            stg[:, s_ : s_ + n_, :],
                            tab[0:HALFROWS, :],
                            idx_sb[:, (coff + s_) * 8 : (coff + s_ + n_) * 8],
                            n_ * P,
                            n_ * P,
                            P,
                            queue_num=_next_q(),
                        )
                    for s_ in range(0, cb, CALL_CAP):
                        n_ = min(CALL_CAP, cb - s_)
                        nc.gpsimd.dma_gather(
                            stg[:, ca + s_ : ca + s_ + n_, :],
                            tab[HALFROWS:ROWS, :],
                            idx_sb[:, (coff + ca + s_) * 8 : (coff + ca + s_ + n_) * 8],
                            n_ * P,
                            n_ * P,
                            P,
                            queue_num=_next_q(),
                        )
                    tail = npacks * 4 - ntot
                    if tail:
                        nc.vector.memset(stg[:, ntot : ntot + tail, :], 0.0)
                    sg = sig_sb[:, p_ * P : (p_ + 1) * P]
                    aps = ps_agg.tile([P, 4 * P], f32, tag="agg")
                    for k in range(npacks):
                        nc.tensor.matmul(
                            aps[:],
                            lhsT=sg,
                            rhs=stg[:, 4 * k : 4 * (k + 1), :].rearrange(
                                "p c f -> p (c f)"
                            ),
                            start=(k == 0),
                            stop=(k == npacks - 1),
                        )
                    drain(layer, p_, aps)
                    coff += ca + cb

            aggregate(0)

            nc.gpsimd.collective_compute(
                "AllGather",
                mybir.AluOpType.bypass,
                replica_groups=[list(range(NCORES))],
                ins=[shard2.opt()],
                outs=[table2.opt()],
            )

            aggregate(1)

    nc.compile()  # bacc passes: library loads, register allocation, DCE
    _split_sync_waits(nc, mybir, max_waits=1)
    return nc


def _split_sync_waits(nc, mybir, max_waits=1):
    """This walrus build rejects instructions with more than `max_waits` sync
    waits; hoist excess waits onto injected same-engine InstNoOps."""
    n_split = 0
    for fn in nc.m.functions:
        for bb in fn.blocks:
            out = []
            changed = False
            for ins in bb.instructions:
                si = ins.sync_info
                if si is not None and si.on_wait and len(si.on_wait) > max_waits:
                    waits = list(si.on_wait)
                    excess = waits[:-max_waits]
                    for i in range(0, len(excess), max_waits):
                        nop = mybir.InstNoOp(
                            name=nc.get_next_instruction_name(),
                            sync_info=mybir.SyncInfo(
                                on_wait=excess[i : i + max_waits], on_update=[]
                            ),
                            bass_nofuse=True,
                            engine=ins.engine,
                        )
                        out.append(nop)
                        n_split += 1
                    si.on_wait = waits[-max_waits:]
                    ins.sync_info = si
                    changed = True
                out.append(ins)
            if changed:
                bb.instructions = out
    return n_split


# ----------------------------------------------------------------------------
# Entry point
# ----------------------------------------------------------------------------
def kernel(x, edge_index, W1, b1, W2, b2):
    global LAST_RESULTS
    from concourse.bass_utils import run_bass_kernel_spmd

    x = np.asarray(x)
    W1a = np.asarray(W1)
    b1a = np.asarray(b1)
    W2a = np.asarray(W2)
    b2a = np.asarray(b2)

    key = hash(np.asarray(edge_index)[:, :: E // 997].tobytes())
    if key not in _CACHE:
        plan = _plan(edge_index)
        nc = _build(plan["T"], plan["CA"], plan["CB"], plan["tot_chunks"])
        _CACHE[key] = (plan, nc)
    plan, nc = _CACHE[key]

    T = plan["T"]
    SLOTS = plan["SLOTS"]

    # xT in table order, tile-major: per core [T, 128 infeat, 128 nodes]
    gpos = plan["pos_of"]  # global table row per node
    xTflat = np.zeros((P, NCORES * SLOTS), dtype=np.float16)
    xTflat[:, gpos] = x.astype(np.float16).T

    in_common = {
        "W1": W1a.astype(np.float16),
        "W2": W2a.astype(np.float16),
        "b1bc": np.broadcast_to(b1a.astype(np.float32), (P, HID)).copy(),
        "b2bc": np.broadcast_to(b2a.astype(np.float32), (P, OUT_CH)).copy(),
        "ident": np.eye(P, dtype=np.float16),
    }
    in_maps = []
    for c in range(NCORES):
        m = dict(in_common)
        m["xTs"] = np.ascontiguousarray(
            xTflat[:, c * SLOTS : (c + 1) * SLOTS].reshape(P, T, P).transpose(1, 0, 2)
        )
        m["sigma"] = plan["sigma_cores"][c]
        m["dinv_own"] = plan["dinv_own_cores"][c]
        m["idx"] = plan["idx_cores"][c]
        in_maps.append(m)

    res = run_bass_kernel_spmd(nc, in_maps, core_ids=list(range(NCORES)))
    LAST_RESULTS = res

    out = np.empty((N, OUT_CH), dtype=np.float32)
    core_of = plan["core_of"]
    slot_of = plan["slot_of"]
    for c in range(NCORES):
        sel = core_of == c
        out[sel] = res.results[c]["out"][slot_of[sel]]
    return out


# revision 9
# speedup vs baseline: 3.1593x; 1.0048x over previous
"""GCN 2-layer encoder on 8 TRN2 NeuronCores (Bass/Tile).

Math (PyG GCNConv, symmetric normalization, self-loops, deg from dst):
    out1 = relu(Dh @ A @ Dh @ (x @ W1) + b1),  Dh = diag(deg^-1/2)
    out  = Dh @ A @ Dh @ (relu1 @ W2) + b2

Factorization used here (per layer):
    table = Dh @ (feat @ W)          # per-node rows, built on device
    agg[d] = sum_{e: src->d} table[src]   (self loops included as edges)
    out[d] = dinv[d] * agg[d] + b

Sharding: nodes are assigned to 8 cores (balanced by in-degree). Each core
builds only its own table shard; shards are exchanged with one AllGather per
layer. Each core aggregates only its own dst nodes: edges of each dst are
packed into SBUF "lanes"; message chunks [128 lanes, F] are fetched with
SWDGE dma_gather (int16 indices, 4 queues round-robin — a single queue
backpressures descriptor generation ~3x) and multiplied by a per-tile
constant sigma [128 lanes, 128 cols] on the PE, accumulating in PSUM.

Since indices are int16, the node table is split in two halves (src cores
0-3 / 4-7) and each (tile, half) run is a separate gather call.
"""

import sys
import types

sys.path.insert(0, "/opt/trn_rl_repo")

import numpy as np

# Register the NTFF profile hook the container's antenv stub lacks, so
# BASS_TRACE=1 profiling works under axon (harmless otherwise).
if "antenv.axon_hooks" not in sys.modules:
    try:
        from trn_agent_boot.trn_boot import _ntff_profile_via_ctypes

        _hook = _ntff_profile_via_ctypes("/opt/axon/libaxon_pjrt.so")
    except Exception:
        _hook = None
    _m = types.ModuleType("antenv.axon_hooks")
    _m.get_axon_ntff_profile_hook = lambda: _hook
    sys.modules["antenv.axon_hooks"] = _m

N = 50000
E = 800000
IN_CH = 128
HID = 128
OUT_CH = 64
NCORES = 8
P = 128
CAP = 15  # max edges per lane per half
CALL_CAP = 8  # max chunks (x128 idxs) per dma_gather call; larger calls fail on HW
SWDGE_QUEUES = 4  # ucode MAX_SWDGE_QUEUES; one queue backpressures desc-gen

_CACHE = {}
LAST_RESULTS = None


# ----------------------------------------------------------------------------
# Host-side planning
# ----------------------------------------------------------------------------
def _plan(edge_index):
    src = np.asarray(edge_index[0], dtype=np.int64)
    dst = np.asarray(edge_index[1], dtype=np.int64)
    loops = np.arange(N, dtype=np.int64)
    src_all = np.concatenate([src, loops])
    dst_all = np.concatenate([dst, loops])
    deg = np.bincount(dst_all, minlength=N)
    dinv = (1.0 / np.sqrt(deg.astype(np.float64))).astype(np.float32)

    # --- node -> core (snake over degree-sorted nodes: balances sum(deg)) ---
    order = np.argsort(-deg, kind="stable")
    snake = np.tile(
        np.concatenate([np.arange(NCORES), np.arange(NCORES - 1, -1, -1)]),
        N // (2 * NCORES) + 1,
    )[:N]
    core_of = np.empty(N, dtype=np.int64)
    core_of[order] = snake

    # --- per-dst A/B in-edge counts (A = src on cores 0-3) ------------------
    isA = core_of[src_all] < (NCORES // 2)
    a_cnt = np.bincount(dst_all[isA], minlength=N)
    b_cnt = np.bincount(dst_all[~isA], minlength=N)

    # --- per-node lanes; per-half fill = chunks a node's lanes need --------
    n_lanes = np.maximum(
        1, np.maximum(-(-a_cnt // CAP), -(-b_cnt // CAP))
    ).astype(np.int64)
    af = -(-a_cnt // n_lanes)
    bf = -(-b_cnt // n_lanes)

    # pack each core's nodes into tiles of <=128 lanes, uniform fill per tile
    core_tiles = []  # per core: list of tiles; tile = list of node ids
    for c in range(NCORES):
        nodes = np.where(core_of == c)[0]
        key = np.maximum(af[nodes], bf[nodes]) * 1000 + np.minimum(
            af[nodes], bf[nodes]
        )
        o2 = np.argsort(-key, kind="stable")
        tiles = []
        cur = []
        cur_lanes = 0
        for i in o2:
            nd = nodes[i]
            nl = n_lanes[nd]
            if cur_lanes + nl > P:
                tiles.append(cur)
                cur = []
                cur_lanes = 0
            cur.append(nd)
            cur_lanes += nl
        if cur:
            tiles.append(cur)
        needs = [
            max(int(af[nd]) for nd in t) + max(int(bf[nd]) for nd in t)
            for t in tiles
        ]
        o3 = sorted(range(len(tiles)), key=lambda i: -needs[i])
        core_tiles.append([tiles[i] for i in o3])

    # global tile count: +1 guarantees an empty last tile on every core
    # (its slot 127 is the guaranteed zero row used for gather padding)
    T = max(len(t) for t in core_tiles) + 1
    SLOTS = T * P
    ROWS = NCORES * SLOTS
    HALF = (NCORES // 2) * SLOTS
    assert HALF <= 32768, (T, SLOTS)

    CA = np.zeros(T, dtype=np.int64)
    CB = np.zeros(T, dtype=np.int64)
    for c in range(NCORES):
        for p_, tile_nodes in enumerate(core_tiles[c]):
            CA[p_] = max(CA[p_], max(int(af[nd]) for nd in tile_nodes))
            CB[p_] = max(CB[p_], max(int(bf[nd]) for nd in tile_nodes))
    zero = (CA + CB) == 0
    CA[zero] = 1  # every tile gets >=1 chunk so PSUM is always initialized
    tot_chunks = int(CA.sum() + CB.sum())

    # --- slot assignment ----------------------------------------------------
    slot_of = np.full(N, -1, dtype=np.int64)
    lane0_of = np.full(N, -1, dtype=np.int64)
    col_of = np.full(N, -1, dtype=np.int64)
    for c in range(NCORES):
        for p_, tile_nodes in enumerate(core_tiles[c]):
            lane = 0
            for col, nd in enumerate(tile_nodes):
                col_of[nd] = col
                lane0_of[nd] = lane
                slot_of[nd] = p_ * P + col
                lane += n_lanes[nd]
            assert lane <= P
    pos_of = core_of * SLOTS + slot_of  # global table row per node

    # --- CSR of edges grouped by (dst, side) -------------------------------
    side = (~isA).astype(np.int64)
    eorder = np.argsort(dst_all * 2 + side, kind="stable")
    src_pos_sorted = pos_of[src_all[eorder]].astype(np.int64)
    estart = np.zeros(N + 1, dtype=np.int64)
    np.cumsum(deg, out=estart[1:])

    # --- gather index arrays + sigma ---------------------------------------
    PAD = HALF - 1  # guaranteed-zero row within each half
    idx_cores = []
    sigma_cores = []
    dinv_own_cores = []
    for c in range(NCORES):
        tiles = core_tiles[c]
        blocksA = [np.full((int(CA[p_]), P), PAD, np.int64) for p_ in range(T)]
        blocksB = [np.full((int(CB[p_]), P), PAD, np.int64) for p_ in range(T)]
        sig = np.zeros((T, P, P), dtype=np.float16)
        dvo = np.zeros((P, T), dtype=np.float32)
        for p_ in range(len(tiles)):
            for nd in tiles[p_]:
                nl = int(n_lanes[nd])
                l0 = int(lane0_of[nd])
                col = int(col_of[nd])
                sig[p_, l0 : l0 + nl, col] = 1.0
                dvo[col, p_] = dinv[nd]
                s0 = int(estart[nd])
                a = int(a_cnt[nd])
                b = int(b_cnt[nd])
                asrc = src_pos_sorted[s0 : s0 + a]
                bsrc = src_pos_sorted[s0 + a : s0 + a + b] - HALF
                for j in range(nl):
                    ach = asrc[j::nl]
                    bch = bsrc[j::nl]
                    if len(ach):
                        blocksA[p_][: len(ach), l0 + j] = ach
                    if len(bch):
                        blocksB[p_][: len(bch), l0 + j] = bch
        flat = []
        for p_ in range(T):
            flat.append(blocksA[p_].reshape(-1))
            flat.append(blocksB[p_].reshape(-1))
        flat = np.concatenate(flat)
        assert flat.size == tot_chunks * P
        assert flat.min() >= 0 and flat.max() < HALF
        wrapped = flat.astype(np.int16).reshape(-1, 16).T.copy()  # [16, n*8]
        idx_cores.append(np.tile(wrapped, (8, 1)))  # replicate to 128 parts
        sigma_cores.append(
            np.ascontiguousarray(sig.transpose(1, 0, 2).reshape(P, T * P))
        )
        dinv_own_cores.append(dvo)

    return dict(
        T=T,
        SLOTS=SLOTS,
        CA=CA,
        CB=CB,
        tot_chunks=tot_chunks,
        core_of=core_of,
        slot_of=slot_of,
        pos_of=pos_of,
        dinv=dinv,
        idx_cores=idx_cores,
        sigma_cores=sigma_cores,
        dinv_own_cores=dinv_own_cores,
    )


# ----------------------------------------------------------------------------
# Device kernel
# ----------------------------------------------------------------------------
def _build(T, CA, CB, tot_chunks):
    import concourse.bass as bass
    import concourse.mybir as mybir
    import concourse.tile as tile
    from concourse import bacc

    f16 = mybir.dt.float16
    f32 = mybir.dt.float32
    i16 = mybir.dt.int16
    SLOTS = T * P
    ROWS = NCORES * SLOTS
    HALFROWS = ROWS // 2
    CTMAX = -(-int(max(CA + CB)) // 4) * 4  # staging chunks, rounded to packs of 4

    nc = bacc.Bacc(
        "TRN2",
        target_bir_lowering=False,
        num_devices=NCORES,
        num_swdge_queues=SWDGE_QUEUES,
    )
    qn = [0]

    def _next_q():
        q = qn[0]
        qn[0] = (qn[0] + 1) % SWDGE_QUEUES
        return q

    xTs_in = nc.dram_tensor("xTs", [T, P, P], f16, kind="ExternalInput")
    w1_in = nc.dram_tensor("W1", [IN_CH, HID], f16, kind="ExternalInput")
    w2_in = nc.dram_tensor("W2", [HID, OUT_CH], f16, kind="ExternalInput")
    b1_in = nc.dram_tensor("b1bc", [P, HID], f32, kind="ExternalInput")
    b2_in = nc.dram_tensor("b2bc", [P, OUT_CH], f32, kind="ExternalInput")
    id_in = nc.dram_tensor("ident", [P, P], f16, kind="ExternalInput")
    sig_in = nc.dram_tensor("sigma", [P, T * P], f16, kind="ExternalInput")
    do_in = nc.dram_tensor("dinv_own", [P, T], f32, kind="ExternalInput")
    idx_in = nc.dram_tensor("idx", [P, tot_chunks * 8], i16, kind="ExternalInput")
    out_ext = nc.dram_tensor("out", [SLOTS, OUT_CH], f32, kind="ExternalOutput")

    with tile.TileContext(nc) as tc:
        with (
            tc.tile_pool(name="const", bufs=1) as cpool,
            tc.tile_pool(name="xt", bufs=4) as xtpool,
            tc.tile_pool(name="stg", bufs=6) as stgpool,
            tc.tile_pool(name="drain", bufs=4) as dpool,
            tc.tile_pool(name="psb", bufs=2, space="PSUM") as ps_build,
            tc.tile_pool(name="psa", bufs=3, space="PSUM") as ps_agg,
            tc.tile_pool(name="pst", bufs=2, space="PSUM") as ps_tr,
            tc.tile_pool(name="psm", bufs=1, space="PSUM") as ps_mm2,
            tc.tile_pool(name="dram", bufs=1, space="DRAM") as dram,
        ):
            # ---- constants into SBUF ----
            w1_sb = cpool.tile([IN_CH, HID], f16)
            nc.sync.dma_start(out=w1_sb[:], in_=w1_in[:])
            w2_sb = cpool.tile([HID, OUT_CH], f16)
            nc.sync.dma_start(out=w2_sb[:], in_=w2_in[:])
            b1_sb = cpool.tile([P, HID], f32)
            nc.sync.dma_start(out=b1_sb[:], in_=b1_in[:])
            b2_sb = cpool.tile([P, OUT_CH], f32)
            nc.sync.dma_start(out=b2_sb[:], in_=b2_in[:])
            id_sb = cpool.tile([P, P], f16)
            nc.sync.dma_start(out=id_sb[:], in_=id_in[:])
            sig_sb = cpool.tile([P, T * P], f16)
            nc.sync.dma_start(out=sig_sb[:], in_=sig_in[:])
            do_sb = cpool.tile([P, T], f32)
            nc.sync.dma_start(out=do_sb[:], in_=do_in[:])
            idx_sb = cpool.tile([P, tot_chunks * 8], i16)
            nc.sync.dma_start(out=idx_sb[:], in_=idx_in[:])

            shard1 = dram.tile([SLOTS, HID], f16)
            table1 = dram.tile([ROWS, HID], f16, addr_space="Shared")
            shard2 = dram.tile([SLOTS, P], f16)
            table2 = dram.tile([ROWS, P], f16, addr_space="Shared")

            # ---- phase 1: own shard of table1 = dinv * (x @ W1) ----
            for j in range(T):
                xt_t = xtpool.tile([P, P], f16, tag="xt")
                nc.sync.dma_start(out=xt_t[:], in_=xTs_in[j])
                bps = ps_build.tile([P, HID], f32, tag="build")
                nc.tensor.matmul(
                    bps[:], lhsT=xt_t[:], rhs=w1_sb[:], start=True, stop=True
                )
                h1t = xtpool.tile([P, HID], f16, tag="h1t")
                nc.scalar.activation(
                    h1t[:],
                    bps[:],
                    mybir.ActivationFunctionType.Copy,
                    scale=do_sb[:, j : j + 1],
                )
                nc.sync.dma_start(out=shard1[j * P : (j + 1) * P, :], in_=h1t[:])

            nc.gpsimd.collective_compute(
                "AllGather",
                mybir.AluOpType.bypass,
                replica_groups=[list(range(NCORES))],
                ins=[shard1.opt()],
                outs=[table1.opt()],
            )

            # ---- per-layer drain + aggregation ----
            def drain(layer, p_, aps):
                # combine the four interleaved partial sums in the wide PSUM
                dv = do_sb[:, p_ : p_ + 1]
                q = dpool.tile([P, P], f32, tag="q")
                nc.vector.tensor_copy(q[:], aps[:, 0:P])
                nc.vector.tensor_add(q[:], q[:], aps[:, P : 2 * P])
                nc.vector.tensor_add(q[:], q[:], aps[:, 2 * P : 3 * P])
                nc.vector.tensor_add(q[:], q[:], aps[:, 3 * P : 4 * P])
                if layer == 0:
                    # r1 = dinv*agg + b1 ; r3 = relu(r1)*dinv (fp16)
                    r1 = dpool.tile([P, HID], f32, tag="r1")
                    nc.scalar.activation(
                        r1[:], q[:], mybir.ActivationFunctionType.Copy, scale=dv
                    )
                    nc.vector.tensor_add(r1[:], r1[:], b1_sb[:])
                    r3 = dpool.tile([P, HID], f16, tag="r3")
                    nc.scalar.activation(
                        r3[:], r1[:], mybir.ActivationFunctionType.Relu, scale=dv
                    )
                    psT = ps_tr.tile([P, P], f16, tag="tr")
                    nc.tensor.transpose(psT[:], r3[:], id_sb[:])
                    rT = dpool.tile([P, P], f16, tag="rT")
                    nc.vector.tensor_copy(rT[:], psT[:])
                    ps2 = ps_mm2.tile([P, OUT_CH], f32, tag="mm2")
                    nc.tensor.matmul(
                        ps2[:], lhsT=rT[:], rhs=w2_sb[:], start=True, stop=True
                    )
                    t2 = dpool.tile([P, P], f16, tag="t2")
                    nc.scalar.activation(
                        t2[:, 0:OUT_CH], ps2[:], mybir.ActivationFunctionType.Copy
                    )
                    nc.vector.memset(t2[:, OUT_CH:P], 0.0)
                    nc.sync.dma_start(out=shard2[p_ * P : (p_ + 1) * P, :], in_=t2[:])
                else:
                    o1 = dpool.tile([P, OUT_CH], f32, tag="o1")
                    nc.scalar.activation(
                        o1[:], q[:, 0:OUT_CH], mybir.ActivationFunctionType.Copy, scale=dv
                    )
                    nc.vector.tensor_add(o1[:], o1[:], b2_sb[:])
                    nc.sync.dma_start(out=out_ext[p_ * P : (p_ + 1) * P, :], in_=o1[:])

            def aggregate(layer):
                tab = table1 if layer == 0 else table2
                coff = 0
                for p_ in range(T):
                    ca = int(CA[p_])
                    cb = int(CB[p_])
                    ntot = ca + cb
                    npacks = -(-ntot // 4)
                    # A chunks at [0, ca), B chunks at [ca, ca+cb)
                    stg = stgpool.tile([P, CTMAX, P], f16, tag="stg")
                    for s_ in range(0, ca, CALL_CAP):
                        n_ = min(CALL_CAP, ca - s_)
                        nc.gpsimd.dma_gather(
                            stg[:, s_ : s_ + n_, :],
                            tab[0:HALFROWS, :],
                            idx_sb[:, (coff + s_) * 8 : (coff + s_ + n_) * 8],
                            n_ * P,
                            n_ * P,
                            P,
                            queue_num=_next_q(),
                        )
                    for s_ in range(0, cb, CALL_CAP):
                        n_ = min(CALL_CAP, cb - s_)
                        nc.gpsimd.dma_gather(
                            stg[:, ca + s_ : ca + s_ + n_, :],
                            tab[HALFROWS:ROWS, :],
                            idx_sb[:, (coff + ca + s_) * 8 : (coff + ca + s_ + n_) * 8],
                            n_ * P,
                            n_ * P,
                            P,
                            queue_num=_next_q(),
                        )
                    tail = npacks * 4 - ntot
                    if tail:
                        nc.vector.memset(stg[:, ntot : ntot + tail, :], 0.0)
                    sg = sig_sb[:, p_ * P : (p_ + 1) * P]
                    aps = ps_agg.tile([P, 4 * P], f32, tag="agg")
                    for k in range(npacks):
                        nc.tensor.matmul(
                            aps[:],
                            lhsT=sg,
                            rhs=stg[:, 4 * k : 4 * (k + 1), :].rearrange(
                                "p c f -> p (c f)"
                            ),
                            start=(k == 0),
                            stop=(k == npacks - 1),
                        )
                    drain(layer, p_, aps)
                    coff += ca + cb

            aggregate(0)

            nc.gpsimd.collective_compute(
                "AllGather",
                mybir.AluOpType.bypass,
                replica_groups=[list(range(NCORES))],
                ins=[shard2.opt()],
                outs=[table2.opt()],
            )

            aggregate(1)

    nc.compile()  # bacc passes: library loads, register allocation, DCE
    _split_sync_waits(nc, mybir, max_waits=1)
    return nc


def _split_sync_waits(nc, mybir, max_waits=1):
    """This walrus build rejects instructions with more than `max_waits` sync
    waits; hoist excess waits onto injected same-engine InstNoOps."""
    n_split = 0
    for fn in nc.m.functions:
        for bb in fn.blocks:
            out = []
            changed = False
            for ins in bb.instructions:
                si = ins.sync_info
                if si is not None and si.on_wait and len(si.on_wait) > max_waits:
                    waits = list(si.on_wait)
                    excess = waits[:-max_waits]
                    for i in range(0, len(excess), max_waits):
                        nop = mybir.InstNoOp(
                            name=nc.get_next_instruction_name(),
                            sync_info=mybir.SyncInfo(
                                on_wait=excess[i : i + max_waits], on_update=[]
                            ),
                            bass_nofuse=True,
                            engine=ins.engine,
                        )
                        out.append(nop)
                        n_split += 1
                    si.on_wait = waits[-max_waits:]
                    ins.sync_info = si
                    changed = True
                out.append(ins)
            if changed:
                bb.instructions = out
    return n_split


# ----------------------------------------------------------------------------
# Entry point
# ----------------------------------------------------------------------------
def kernel(x, edge_index, W1, b1, W2, b2):
    global LAST_RESULTS
    from concourse.bass_utils import run_bass_kernel_spmd

    x = np.asarray(x)
    W1a = np.asarray(W1)
    b1a = np.asarray(b1)
    W2a = np.asarray(W2)
    b2a = np.asarray(b2)

    key = hash(np.asarray(edge_index)[:, :: E // 997].tobytes())
    if key not in _CACHE:
        plan = _plan(edge_index)
        nc = _build(plan["T"], plan["CA"], plan["CB"], plan["tot_chunks"])
        _CACHE[key] = (plan, nc)
    plan, nc = _CACHE[key]

    T = plan["T"]
    SLOTS = plan["SLOTS"]

    # xT in table order, tile-major: per core [T, 128 infeat, 128 nodes]
    gpos = plan["pos_of"]  # global table row per node
    xTflat = np.zeros((P, NCORES * SLOTS), dtype=np.float16)
    xTflat[:, gpos] = x.astype(np.float16).T

    in_common = {
        "W1": W1a.astype(np.float16),
        "W2": W2a.astype(np.float16),
        "b1bc": np.broadcast_to(b1a.astype(np.float32), (P, HID)).copy(),
        "b2bc": np.broadcast_to(b2a.astype(np.float32), (P, OUT_CH)).copy(),
        "ident": np.eye(P, dtype=np.float16),
    }
    in_maps = []
    for c in range(NCORES):
        m = dict(in_common)
        m["xTs"] = np.ascontiguousarray(
            xTflat[:, c * SLOTS : (c + 1) * SLOTS].reshape(P, T, P).transpose(1, 0, 2)
        )
        m["sigma"] = plan["sigma_cores"][c]
        m["dinv_own"] = plan["dinv_own_cores"][c]
        m["idx"] = plan["idx_cores"][c]
        in_maps.append(m)

    res = run_bass_kernel_spmd(nc, in_maps, core_ids=list(range(NCORES)))
    LAST_RESULTS = res

    out = np.empty((N, OUT_CH), dtype=np.float32)
    core_of = plan["core_of"]
    slot_of = plan["slot_of"]
    for c in range(NCORES):
        sel = core_of == c
        out[sel] = res.results[c]["out"][slot_of[sel]]
    return out


# revision 13
# speedup vs baseline: 3.3179x; 1.0502x over previous
"""GCN 2-layer encoder on 8 TRN2 NeuronCores (Bass/Tile).

Math (PyG GCNConv, symmetric normalization, self-loops, deg from dst):
    out1 = relu(Dh @ A @ Dh @ (x @ W1) + b1),  Dh = diag(deg^-1/2)
    out  = Dh @ A @ Dh @ (relu1 @ W2) + b2

Factorization used here (per layer):
    table = Dh @ (feat @ W)          # per-node rows, built on device
    agg[d] = sum_{e: src->d} table[src]   (self loops included as edges)
    out[d] = dinv[d] * agg[d] + b

Sharding: nodes are assigned to 8 cores (balanced by in-degree). Each core
builds only its own table shard; shards are exchanged with one AllGather per
layer. Each core aggregates only its own dst nodes: edges of each dst are
packed into SBUF "lanes"; message chunks [128 lanes, F] are fetched with
SWDGE dma_gather (int16 indices, 4 queues round-robin — a single queue
backpressures descriptor generation ~3x) and multiplied by a per-tile
constant sigma [128 lanes, 128 cols] on the PE, accumulating in PSUM.

Since indices are int16, the node table is split in two halves (src cores
0-3 / 4-7) and each (tile, half) run is a separate gather call.
"""

import sys
import types

sys.path.insert(0, "/opt/trn_rl_repo")

import numpy as np

# Register the NTFF profile hook the container's antenv stub lacks, so
# BASS_TRACE=1 profiling works under axon (harmless otherwise).
if "antenv.axon_hooks" not in sys.modules:
    try:
        from trn_agent_boot.trn_boot import _ntff_profile_via_ctypes

        _hook = _ntff_profile_via_ctypes("/opt/axon/libaxon_pjrt.so")
    except Exception:
        _hook = None
    _m = types.ModuleType("antenv.axon_hooks")
    _m.get_axon_ntff_profile_hook = lambda: _hook
    sys.modules["antenv.axon_hooks"] = _m

N = 50000
E = 800000
IN_CH = 128
HID = 128
OUT_CH = 64
NCORES = 8
P = 128
CAP = 15  # max edges per lane per half
CALL_CAP = 8  # max chunks (x128 idxs) per dma_gather call; larger calls fail on HW
SWDGE_QUEUES = 4  # ucode MAX_SWDGE_QUEUES; one queue backpressures desc-gen

_CACHE = {}
LAST_RESULTS = None


# ----------------------------------------------------------------------------
# Host-side planning
# ----------------------------------------------------------------------------
def _plan(edge_index):
    src = np.asarray(edge_index[0], dtype=np.int64)
    dst = np.asarray(edge_index[1], dtype=np.int64)
    loops = np.arange(N, dtype=np.int64)
    src_all = np.concatenate([src, loops])
    dst_all = np.concatenate([dst, loops])
    deg = np.bincount(dst_all, minlength=N)
    dinv = (1.0 / np.sqrt(deg.astype(np.float64))).astype(np.float32)

    # --- node -> core (snake over degree-sorted nodes: balances sum(deg)) ---
    order = np.argsort(-deg, kind="stable")
    snake = np.tile(
        np.concatenate([np.arange(NCORES), np.arange(NCORES - 1, -1, -1)]),
        N // (2 * NCORES) + 1,
    )[:N]
    core_of = np.empty(N, dtype=np.int64)
    core_of[order] = snake

    # --- per-dst A/B in-edge counts (A = src on cores 0-3) ------------------
    isA = core_of[src_all] < (NCORES // 2)
    a_cnt = np.bincount(dst_all[isA], minlength=N)
    b_cnt = np.bincount(dst_all[~isA], minlength=N)

    # --- per-node lanes; per-half fill = chunks a node's lanes need --------
    n_lanes = np.maximum(
        1, np.maximum(-(-a_cnt // CAP), -(-b_cnt // CAP))
    ).astype(np.int64)
    af = -(-a_cnt // n_lanes)
    bf = -(-b_cnt // n_lanes)

    # pack each core's nodes into tiles of <=128 lanes, uniform fill per tile
    core_tiles = []  # per core: list of tiles; tile = list of node ids
    for c in range(NCORES):
        nodes = np.where(core_of == c)[0]
        key = np.maximum(af[nodes], bf[nodes]) * 1000 + np.minimum(
            af[nodes], bf[nodes]
        )
        o2 = np.argsort(-key, kind="stable")
        tiles = []
        cur = []
        cur_lanes = 0
        for i in o2:
            nd = nodes[i]
            nl = n_lanes[nd]
            if cur_lanes + nl > P:
                tiles.append(cur)
                cur = []
                cur_lanes = 0
            cur.append(nd)
            cur_lanes += nl
        if cur:
            tiles.append(cur)
        needs = [
            max(int(af[nd]) for nd in t) + max(int(bf[nd]) for nd in t)
            for t in tiles
        ]
        o3 = sorted(range(len(tiles)), key=lambda i: -needs[i])
        core_tiles.append([tiles[i] for i in o3])

    # global tile count: +1 guarantees an empty last tile on every core
    # (its slot 127 is the guaranteed zero row used for gather padding)
    T = max(len(t) for t in core_tiles) + 1
    SLOTS = T * P
    ROWS = NCORES * SLOTS
    HALF = (NCORES // 2) * SLOTS
    assert HALF <= 32768, (T, SLOTS)

    CA = np.zeros(T, dtype=np.int64)
    CB = np.zeros(T, dtype=np.int64)
    for c in range(NCORES):
        for p_, tile_nodes in enumerate(core_tiles[c]):
            CA[p_] = max(CA[p_], max(int(af[nd]) for nd in tile_nodes))
            CB[p_] = max(CB[p_], max(int(bf[nd]) for nd in tile_nodes))
    zero = (CA + CB) == 0
    CA[zero] = 1  # every tile gets >=1 chunk so PSUM is always initialized
    tot_chunks = int(CA.sum() + CB.sum())

    # --- slot assignment ----------------------------------------------------
    slot_of = np.full(N, -1, dtype=np.int64)
    lane0_of = np.full(N, -1, dtype=np.int64)
    col_of = np.full(N, -1, dtype=np.int64)
    for c in range(NCORES):
        for p_, tile_nodes in enumerate(core_tiles[c]):
            lane = 0
            for col, nd in enumerate(tile_nodes):
                col_of[nd] = col
                lane0_of[nd] = lane
                slot_of[nd] = p_ * P + col
                lane += n_lanes[nd]
            assert lane <= P
    pos_of = core_of * SLOTS + slot_of  # global table row per node

    # --- CSR of edges grouped by (dst, side) -------------------------------
    side = (~isA).astype(np.int64)
    eorder = np.argsort(dst_all * 2 + side, kind="stable")
    src_pos_sorted = pos_of[src_all[eorder]].astype(np.int64)
    estart = np.zeros(N + 1, dtype=np.int64)
    np.cumsum(deg, out=estart[1:])

    # --- gather index arrays + sigma ---------------------------------------
    PAD = HALF - 1  # guaranteed-zero row within each half
    idx_cores = []
    sigma_cores = []
    dinv_own_cores = []
    for c in range(NCORES):
        tiles = core_tiles[c]
        blocksA = [np.full((int(CA[p_]), P), PAD, np.int64) for p_ in range(T)]
        blocksB = [np.full((int(CB[p_]), P), PAD, np.int64) for p_ in range(T)]
        sig = np.zeros((T, P, P), dtype=np.float16)
        dvo = np.zeros((P, T), dtype=np.float32)
        for p_ in range(len(tiles)):
            for nd in tiles[p_]:
                nl = int(n_lanes[nd])
                l0 = int(lane0_of[nd])
                col = int(col_of[nd])
                sig[p_, l0 : l0 + nl, col] = 1.0
                dvo[col, p_] = dinv[nd]
                s0 = int(estart[nd])
                a = int(a_cnt[nd])
                b = int(b_cnt[nd])
                asrc = src_pos_sorted[s0 : s0 + a]
                bsrc = src_pos_sorted[s0 + a : s0 + a + b] - HALF
                for j in range(nl):
                    ach = asrc[j::nl]
                    bch = bsrc[j::nl]
                    if len(ach):
                        blocksA[p_][: len(ach), l0 + j] = ach
                    if len(bch):
                        blocksB[p_][: len(bch), l0 + j] = bch
        flat = []
        for p_ in range(T):
            flat.append(blocksA[p_].reshape(-1))
            flat.append(blocksB[p_].reshape(-1))
        flat = np.concatenate(flat)
        assert flat.size == tot_chunks * P
        assert flat.min() >= 0 and flat.max() < HALF
        wrapped = flat.astype(np.int16).reshape(-1, 16).T.copy()  # [16, n*8]
        idx_cores.append(np.tile(wrapped, (8, 1)))  # replicate to 128 parts
        sigma_cores.append(
            np.ascontiguousarray(sig.transpose(1, 0, 2).reshape(P, T * P))
        )
        dinv_own_cores.append(dvo)

    return dict(
        T=T,
        SLOTS=SLOTS,
        CA=CA,
        CB=CB,
        tot_chunks=tot_chunks,
        core_of=core_of,
        slot_of=slot_of,
        pos_of=pos_of,
        dinv=dinv,
        idx_cores=idx_cores,
        sigma_cores=sigma_cores,
        dinv_own_cores=dinv_own_cores,
    )


# ----------------------------------------------------------------------------
# Device kernel
# ----------------------------------------------------------------------------
def _build(T, CA, CB, tot_chunks):
    import concourse.bass as bass
    import concourse.mybir as mybir
    import concourse.tile as tile
    from concourse import bacc

    f16 = mybir.dt.float16
    f32 = mybir.dt.float32
    i16 = mybir.dt.int16
    SLOTS = T * P
    ROWS = NCORES * SLOTS
    HALFROWS = ROWS // 2
    CTMAX = -(-int(max(CA + CB)) // 4) * 4  # staging chunks, rounded to packs of 4

    nc = bacc.Bacc(
        "TRN2",
        target_bir_lowering=False,
        num_devices=NCORES,
        num_swdge_queues=SWDGE_QUEUES,
    )
    qn = [0]

    def _next_q():
        q = qn[0]
        qn[0] = (qn[0] + 1) % SWDGE_QUEUES
        return q

    xTs_in = nc.dram_tensor("xTs", [P, SLOTS], f16, kind="ExternalInput")
    w1_in = nc.dram_tensor("W1", [IN_CH, HID], f16, kind="ExternalInput")
    w2_in = nc.dram_tensor("W2", [HID, OUT_CH], f16, kind="ExternalInput")
    b1_in = nc.dram_tensor("b1bc", [P, HID], f32, kind="ExternalInput")
    b2_in = nc.dram_tensor("b2bc", [P, OUT_CH], f32, kind="ExternalInput")
    id_in = nc.dram_tensor("ident", [P, P], f16, kind="ExternalInput")
    sig_in = nc.dram_tensor("sigma", [P, T * P], f16, kind="ExternalInput")
    do_in = nc.dram_tensor("dinv_own", [P, T], f32, kind="ExternalInput")
    idx_in = nc.dram_tensor("idx", [P, tot_chunks * 8], i16, kind="ExternalInput")
    out_ext = nc.dram_tensor("out", [SLOTS, OUT_CH], f32, kind="ExternalOutput")

    with tile.TileContext(nc) as tc:
        with (
            tc.tile_pool(name="const", bufs=1) as cpool,
            tc.tile_pool(name="xt", bufs=4) as xtpool,
            tc.tile_pool(name="stg", bufs=6) as stgpool,
            tc.tile_pool(name="drain", bufs=4) as dpool,
            tc.tile_pool(name="psb", bufs=2, space="PSUM") as ps_build,
            tc.tile_pool(name="psa", bufs=3, space="PSUM") as ps_agg,
            tc.tile_pool(name="pst", bufs=2, space="PSUM") as ps_tr,
            tc.tile_pool(name="psm", bufs=1, space="PSUM") as ps_mm2,
            tc.tile_pool(name="dram", bufs=1, space="DRAM") as dram,
        ):
            # ---- constants into SBUF ----
            w1_sb = cpool.tile([IN_CH, HID], f16)
            nc.sync.dma_start(out=w1_sb[:], in_=w1_in[:])
            w2_sb = cpool.tile([HID, OUT_CH], f16)
            nc.sync.dma_start(out=w2_sb[:], in_=w2_in[:])
            b1_sb = cpool.tile([P, HID], f32)
            nc.sync.dma_start(out=b1_sb[:], in_=b1_in[:])
            b2_sb = cpool.tile([P, OUT_CH], f32)
            nc.sync.dma_start(out=b2_sb[:], in_=b2_in[:])
            id_sb = cpool.tile([P, P], f16)
            nc.sync.dma_start(out=id_sb[:], in_=id_in[:])
            sig_sb = cpool.tile([P, T * P], f16)
            nc.sync.dma_start(out=sig_sb[:], in_=sig_in[:])
            do_sb = cpool.tile([P, T], f32)
            nc.sync.dma_start(out=do_sb[:], in_=do_in[:])
            idx_sb = cpool.tile([P, tot_chunks * 8], i16)
            nc.sync.dma_start(out=idx_sb[:], in_=idx_in[:])
            xall_sb = cpool.tile([P, SLOTS], f16)
            nc.sync.dma_start(out=xall_sb[:], in_=xTs_in[:])

            shard1 = dram.tile([SLOTS, HID], f16)
            table1 = dram.tile([ROWS, HID], f16, addr_space="Shared")
            shard2 = dram.tile([SLOTS, P], f16)
            table2 = dram.tile([ROWS, P], f16, addr_space="Shared")

            # ---- phase 1: own shard of table1 = dinv * (x @ W1) ----
            for j in range(T):
                bps = ps_build.tile([P, HID], f32, tag="build")
                nc.tensor.matmul(
                    bps[:],
                    lhsT=xall_sb[:, j * P : (j + 1) * P],
                    rhs=w1_sb[:],
                    start=True,
                    stop=True,
                )
                h1t = xtpool.tile([P, HID], f16, tag="h1t")
                nc.scalar.activation(
                    h1t[:],
                    bps[:],
                    mybir.ActivationFunctionType.Copy,
                    scale=do_sb[:, j : j + 1],
                )
                nc.sync.dma_start(out=shard1[j * P : (j + 1) * P, :], in_=h1t[:])

            nc.gpsimd.collective_compute(
                "AllGather",
                mybir.AluOpType.bypass,
                replica_groups=[list(range(NCORES))],
                ins=[shard1.opt()],
                outs=[table1.opt()],
            )

            # ---- per-layer drain + aggregation ----
            def drain(layer, p_, aps):
                # combine the four interleaved partial sums in the wide PSUM
                dv = do_sb[:, p_ : p_ + 1]
                q = dpool.tile([P, P], f32, tag="q")
                nc.vector.tensor_copy(q[:], aps[:, 0:P])
                nc.vector.tensor_add(q[:], q[:], aps[:, P : 2 * P])
                nc.vector.tensor_add(q[:], q[:], aps[:, 2 * P : 3 * P])
                nc.vector.tensor_add(q[:], q[:], aps[:, 3 * P : 4 * P])
                if layer == 0:
                    # r1 = dinv*agg + b1 ; r3 = relu(r1)*dinv (fp16)
                    r1 = dpool.tile([P, HID], f32, tag="r1")
                    nc.scalar.activation(
                        r1[:], q[:], mybir.ActivationFunctionType.Copy, scale=dv
                    )
                    nc.vector.tensor_add(r1[:], r1[:], b1_sb[:])
                    r3 = dpool.tile([P, HID], f16, tag="r3")
                    nc.scalar.activation(
                        r3[:], r1[:], mybir.ActivationFunctionType.Relu, scale=dv
                    )
                    psT = ps_tr.tile([P, P], f16, tag="tr")
                    nc.tensor.transpose(psT[:], r3[:], id_sb[:])
                    rT = dpool.tile([P, P], f16, tag="rT")
                    nc.vector.tensor_copy(rT[:], psT[:])
                    ps2 = ps_mm2.tile([P, OUT_CH], f32, tag="mm2")
                    nc.tensor.matmul(
                        ps2[:], lhsT=rT[:], rhs=w2_sb[:], start=True, stop=True
                    )
                    t2 = dpool.tile([P, P], f16, tag="t2")
                    nc.scalar.activation(
                        t2[:, 0:OUT_CH], ps2[:], mybir.ActivationFunctionType.Copy
                    )
                    nc.vector.memset(t2[:, OUT_CH:P], 0.0)
                    nc.sync.dma_start(out=shard2[p_ * P : (p_ + 1) * P, :], in_=t2[:])
                else:
                    o1 = dpool.tile([P, OUT_CH], f32, tag="o1")
                    nc.scalar.activation(
                        o1[:], q[:, 0:OUT_CH], mybir.ActivationFunctionType.Copy, scale=dv
                    )
                    nc.vector.tensor_add(o1[:], o1[:], b2_sb[:])
                    nc.sync.dma_start(out=out_ext[p_ * P : (p_ + 1) * P, :], in_=o1[:])

            def aggregate(layer):
                tab = table1 if layer == 0 else table2
                coff = 0
                for p_ in range(T):
                    ca = int(CA[p_])
                    cb = int(CB[p_])
                    ntot = ca + cb
                    npacks = -(-ntot // 4)
                    # A chunks at [0, ca), B chunks at [ca, ca+cb)
                    stg = stgpool.tile([P, CTMAX, P], f16, tag="stg")
                    for s_ in range(0, ca, CALL_CAP):
                        n_ = min(CALL_CAP, ca - s_)
                        nc.gpsimd.dma_gather(
                            stg[:, s_ : s_ + n_, :],
                            tab[0:HALFROWS, :],
                            idx_sb[:, (coff + s_) * 8 : (coff + s_ + n_) * 8],
                            n_ * P,
                            n_ * P,
                            P,
                            queue_num=_next_q(),
                        )
                    for s_ in range(0, cb, CALL_CAP):
                        n_ = min(CALL_CAP, cb - s_)
                        nc.gpsimd.dma_gather(
                            stg[:, ca + s_ : ca + s_ + n_, :],
                            tab[HALFROWS:ROWS, :],
                            idx_sb[:, (coff + ca + s_) * 8 : (coff + ca + s_ + n_) * 8],
                            n_ * P,
                            n_ * P,
                            P,
                            queue_num=_next_q(),
                        )
                    tail = npacks * 4 - ntot
                    if tail:
                        nc.vector.memset(stg[:, ntot : ntot + tail, :], 0.0)
                    sg = sig_sb[:, p_ * P : (p_ + 1) * P]
                    aps = ps_agg.tile([P, 4 * P], f32, tag="agg")
                    for k in range(npacks):
                        nc.tensor.matmul(
                            aps[:],
                            lhsT=sg,
                            rhs=stg[:, 4 * k : 4 * (k + 1), :].rearrange(
                                "p c f -> p (c f)"
                            ),
                            start=(k == 0),
                            stop=(k == npacks - 1),
                        )
                    drain(layer, p_, aps)
                    coff += ca + cb

            aggregate(0)

            nc.gpsimd.collective_compute(
                "AllGather",
                mybir.AluOpType.bypass,
                replica_groups=[list(range(NCORES))],
                ins=[shard2.opt()],
                outs=[table2.opt()],
            )

            aggregate(1)

    nc.compile()  # bacc passes: library loads, register allocation, DCE
    _split_sync_waits(nc, mybir, max_waits=1)
    return nc


def _split_sync_waits(nc, mybir, max_waits=1):
    """This walrus build rejects instructions with more than `max_waits` sync
    waits; hoist excess waits onto injected same-engine InstNoOps."""
    n_split = 0
    for fn in nc.m.functions:
        for bb in fn.blocks:
            out = []
            changed = False
            for ins in bb.instructions:
                si = ins.sync_info
                if si is not None and si.on_wait and len(si.on_wait) > max_waits:
                    waits = list(si.on_wait)
                    excess = waits[:-max_waits]
                    for i in range(0, len(excess), max_waits):
                        nop = mybir.InstNoOp(
                            name=nc.get_next_instruction_name(),
                            sync_info=mybir.SyncInfo(
                                on_wait=excess[i : i + max_waits], on_update=[]
                            ),
                            bass_nofuse=True,
                            engine=ins.engine,
                        )
                        out.append(nop)
                        n_split += 1
                    si.on_wait = waits[-max_waits:]
                    ins.sync_info = si
                    changed = True
                out.append(ins)
            if changed:
                bb.instructions = out
    return n_split


# ----------------------------------------------------------------------------
# Entry point
# ----------------------------------------------------------------------------
def kernel(x, edge_index, W1, b1, W2, b2):
    global LAST_RESULTS
    from concourse.bass_utils import run_bass_kernel_spmd

    x = np.asarray(x)
    W1a = np.asarray(W1)
    b1a = np.asarray(b1)
    W2a = np.asarray(W2)
    b2a = np.asarray(b2)

    key = hash(np.asarray(edge_index)[:, :: E // 997].tobytes())
    if key not in _CACHE:
        plan = _plan(edge_index)
        nc = _build(plan["T"], plan["CA"], plan["CB"], plan["tot_chunks"])
        _CACHE[key] = (plan, nc)
    plan, nc = _CACHE[key]

    T = plan["T"]
    SLOTS = plan["SLOTS"]

    # xT in table order, tile-major: per core [T, 128 infeat, 128 nodes]
    gpos = plan["pos_of"]  # global table row per node
    xTflat = np.zeros((P, NCORES * SLOTS), dtype=np.float16)
    xTflat[:, gpos] = x.astype(np.float16).T

    in_common = {
        "W1": W1a.astype(np.float16),
        "W2": W2a.astype(np.float16),
        "b1bc": np.broadcast_to(b1a.astype(np.float32), (P, HID)).copy(),
        "b2bc": np.broadcast_to(b2a.astype(np.float32), (P, OUT_CH)).copy(),
        "ident": np.eye(P, dtype=np.float16),
    }
    in_maps = []
    for c in range(NCORES):
        m = dict(in_common)
        m["xTs"] = np.ascontiguousarray(xTflat[:, c * SLOTS : (c + 1) * SLOTS])
        m["sigma"] = plan["sigma_cores"][c]
        m["dinv_own"] = plan["dinv_own_cores"][c]
        m["idx"] = plan["idx_cores"][c]
        in_maps.append(m)

    res = run_bass_kernel_spmd(nc, in_maps, core_ids=list(range(NCORES)))
    LAST_RESULTS = res

    out = np.empty((N, OUT_CH), dtype=np.float32)
    core_of = plan["core_of"]
    slot_of = plan["slot_of"]
    for c in range(NCORES):
        sel = core_of == c
        out[sel] = res.results[c]["out"][slot_of[sel]]
    return out
